# revision 32
# baseline (speedup 1.0000x reference)
"""Trainium2 Bass kernel for GatedCrossAttention (B=4, N=4096, C=1024, H=16, M=4).

Reference math (dead code removed: the v/gate projections are overwritten
by views of k in the original module, so v = g = k):
    q = query @ Wq.T + bq                    [B,N,C]   -> [B,N,H,hd]
    k = key   @ Wk.T + bk                    [B,N,M,C] -> [B,N,M,H,hd]
    attn = softmax_M(SCALE * einsum('bnhc,bnmhc->bnmh', q, k))
    out  = einsum('bnmh,bnmhc->bnhc', attn, k*k) . reshape(B,N,C)
    out  = out @ Wo.T + bo

Strategy: pure data parallel over the 16384 tokens (8 cores x 2048), no
collectives.  Channels live on partitions, tokens on the free axis, so every
matmul contraction is a natural PE op.

v3 (vs 652us baseline, 541us v2):
  * All matmuls N=512 moving (the per-MM overhead is ~24ns regardless of N,
    so wider is strictly better): k-proj streams an m-pair per MM, q-proj
    batches two token blocks per MM, logits/broadcast stream m-pairs.
  * Logits via a SHARED stationary indicator indl[128,16] per r-tile,
    landing logits head-major [16h, t] in PSUM with 8-way accumulation over
    r-tiles.  No transposes, no tiny-N matmuls.
  * Softmax weights return to channel-major via indbT[16,128] stationary
    matmuls interleaved into the NEXT block's projection loop, so the PE
    stream never waits on the softmax chain (keeps the HAM clock at 2.4GHz;
    the baseline oscillated to 1.2GHz every block).
  * qp*kp and kp^2 are computed straight out of the projection PSUM banks.
  * Weight DMAs are chunked per r-tile on a separate queue (gpsimd) from
    the input DMAs (sync), so the first matmul issues ~3us in instead of
    ~30us.
Accumulation stays f32 in PSUM; activations/weights fp16 (rel err ~1e-3).
"""

import dataclasses
import numpy as np
from contextlib import ExitStack

try:
    import concourse.bass as bass
except ImportError:  # path fallback for bare containers
    import sys

    sys.path.insert(0, "/opt/trn_rl_repo")
    import concourse.bass as bass

import concourse.tile as tile
from concourse import bacc, mybir
from concourse.bass_utils import run_bass_kernel_spmd

# problem constants (hardcoded per the task contract)
B, N, C, H, HD, M = 4, 4096, 1024, 16, 64, 4
SCALE = float(HD) ** -0.5
NCORES = 8
T_TOTAL = B * N
T_CORE = T_TOTAL // NCORES  # 2048
TB = 256                    # tokens per block
NJ = C // 128               # 8 channel tiles (r/c run over these)
NT = TB // 128              # 2 token subtiles per block (for out proj)

DT = mybir.dt.float16
NPDT = np.float16
F32 = mybir.dt.float32
Exp = mybir.ActivationFunctionType.Exp


def _bcast(ap, reps, axis):
    """Insert a 0-stride dim of size `reps` at AP position `axis` (0=partition)."""
    new = list(ap.ap)
    new.insert(axis, [0, reps])
    return dataclasses.replace(ap, ap=new)


def build_nc(t_core=T_CORE, with_bias=False):
    nblk = t_core // TB
    assert nblk % 2 == 0
    nc = bacc.Bacc("TRN2", target_bir_lowering=False, debug=False)

    qT = nc.declare_dram_parameter("qT", [C, t_core], DT, isOutput=False)
    # k pre-shuffled on host to [pair, c-chunk, m-in-pair, partition, t] so a
    # per-pair load is a 3-dim DMA landing [p, c, m, t] in SBUF
    kT = nc.declare_dram_parameter("kT", [2, NJ, 2, 128, t_core], DT,
                                   isOutput=False)
    # wq/wk pre-shuffled on host to [p, r, c, j] so a per-r chunk is a
    # contiguous 2KB-per-partition DMA (256B rows would run at ~36GB/s)
    wqT = nc.declare_dram_parameter("wqT", [128, NJ, NJ, 128], DT,
                                    isOutput=False)
    wkT = nc.declare_dram_parameter("wkT", [128, NJ, NJ, 128], DT,
                                    isOutput=False)
    woT = nc.declare_dram_parameter("woT", [C, C], DT, isOutput=False)
    indl = nc.declare_dram_parameter("indl", [128, NJ, 128], DT, isOutput=False)
    indb = nc.declare_dram_parameter("indb", [128, NJ, 128], DT, isOutput=False)
    if with_bias:
        bq = nc.declare_dram_parameter("bq", [1, C], DT, isOutput=False)
        bk = nc.declare_dram_parameter("bk", [1, C], DT, isOutput=False)
        bo = nc.declare_dram_parameter("bo", [1, C], DT, isOutput=False)
    out = nc.declare_dram_parameter("out", [t_core, C], F32, isOutput=True)

    # DRAM views: channel dim split into (chunk, partition)
    qT_v = qT.ap().rearrange("(c p) t -> p c t", p=128)
    kT_v = kT.ap().rearrange("w c m p t -> w p c m t")
    wo_v = woT.ap().rearrange("(c p) j -> p c j", p=128)

    with tile.TileContext(nc) as tc, ExitStack() as ctx:
        consts = ctx.enter_context(tc.tile_pool(name="consts", bufs=1))
        p_inq = ctx.enter_context(tc.tile_pool(name="inq", bufs=2))
        p_ink = ctx.enter_context(tc.tile_pool(name="ink", bufs=4))
        p_qp = ctx.enter_context(tc.tile_pool(name="qp", bufs=2))
        p_prod = ctx.enter_context(tc.tile_pool(name="prod", bufs=1))
        p_ksq = ctx.enter_context(tc.tile_pool(name="ksq", bufs=2))
        p_sm = ctx.enter_context(tc.tile_pool(name="sm", bufs=2))
        p_ct = ctx.enter_context(tc.tile_pool(name="ct", bufs=2))
        p_y = ctx.enter_context(tc.tile_pool(name="y", bufs=2))
        p_out = ctx.enter_context(tc.tile_pool(name="outs", bufs=4))
        # PSUM (8 banks): pa = q-acc/logits-acc/outproj-acc, pk = k-acc,
        # pw = softmax-weight broadcast
        pa = ctx.enter_context(tc.tile_pool(name="pa", bufs=3, space="PSUM"))
        pk = ctx.enter_context(tc.tile_pool(name="pk", bufs=2, space="PSUM"))
        pw = ctx.enter_context(tc.tile_pool(name="pw", bufs=3, space="PSUM"))

        # ---- weights / constants, chunked per r-tile across DMA queues so
        # the first projection can issue ~6us in: r0 chunks ride ahead of
        # the inputs on sync/vector; the rest trail on gpsimd behind the
        # first k-pair load ----
        wq_sb = consts.tile([128, NJ, NJ, 128], DT)  # [p, r, c, j]
        wk_sb = consts.tile([128, NJ, NJ, 128], DT)
        wo_sb = consts.tile([128, NJ, C], DT)

        def w_chunk(r, eng):
            eng.dma_start(out=wq_sb[:, r], in_=wqT.ap()[:, r])
            eng.dma_start(out=wk_sb[:, r], in_=wkT.ap()[:, r])

        w_chunk(0, nc.sync)
        indl_sb = consts.tile([128, NJ, 128], DT)
        indb_sb = consts.tile([128, NJ, 128], DT)

        def late_consts():
            # spread the remaining weight chunks over all three DMA queues so
            # no queue delays the next block's inputs by a multi-us lump
            for r, eng in [(1, nc.scalar), (2, nc.gpsimd), (3, nc.sync),
                           (4, nc.scalar), (5, nc.gpsimd), (6, nc.sync),
                           (7, nc.scalar)]:
                w_chunk(r, eng)
            nc.gpsimd.dma_start(out=indl_sb, in_=indl.ap())
            nc.gpsimd.dma_start(out=indb_sb, in_=indb.ap())
            nc.sync.dma_start(out=wo_sb, in_=wo_v)
            if with_bias:
                nc.sync.dma_start(out=bq_sb, in_=bq.ap())
                nc.sync.dma_start(out=bk_sb, in_=bk.ap())
                nc.sync.dma_start(out=bo_sb, in_=bo.ap())

        if with_bias:
            ones_sb = consts.tile([1, 2 * TB], DT)
            nc.vector.memset(ones_sb, 1.0)
            bq_sb = consts.tile([1, C], DT)
            bk_sb = consts.tile([1, C], DT)
            bo_sb = consts.tile([1, C], DT)
        else:
            ones_sb = bq_sb = bk_sb = bo_sb = None

        # ---- PE warmup: dummy matmuls on the first weight chunk fill the
        # initial input-DMA wait and flip the HAM clock gate to 2.4GHz
        # before the real stream starts (results are never read) ----
        for i in range(18):
            warm_ps = pw.tile([128, 2, TB], F32, tag="pw", name="warm",
                              padded_shape=[128, 2, 256])
            nc.tensor.matmul(warm_ps, wq_sb[:, 0, 0, :],
                             wk_sb[:, 0, 4 * (i % 2):4 * (i % 2) + 4, :],
                             start=True, stop=True)

        # cross-block pipeline state: (ksq, w_sb, y) of the previous block
        prev = None
        qp = None
        q_in = None

        def emit_tail_pair(pv, r, pr, ctt):
            """PE broadcast MM of one m-pair of the prev block + chasing
            DVE ct mul.  Emitted right after this r-iter's k-pair group so
            the DVE consumer runs ~2us ahead of the slot's next writer."""
            ksq_p, w_p, _ = pv
            ms = slice(2 * pr, 2 * pr + 2)
            wb = pw.tile([128, 2, TB], F32, tag="pw", name="wb",
                         padded_shape=[128, 2, 256])
            nc.tensor.matmul(wb, indb_sb[:, r, :],
                             w_p[:, ms, :], start=True, stop=True)
            nc.vector.tensor_mul(ctt[:, ms, :], wb, ksq_p[:, r, ms, :])

        def emit_tail_adds(pv, r, ctt, eng):
            # tree-add on the otherwise idle GpSimd engine (SBUF-only ops)
            _, _, y_p = pv
            a01 = p_ct.tile([128, TB], DT, tag="a01", name="a01")
            a23 = p_ct.tile([128, TB], DT, tag="a23", name="a23")
            eng.tensor_add(a01, ctt[:, 0, :], ctt[:, 1, :])
            eng.tensor_add(a23, ctt[:, 2, :], ctt[:, 3, :])
            eng.tensor_add(y_p[:, r, :], a01, a23)

        def emit_outproj(pv, blk):
            _, _, y_p = pv
            t0 = blk * TB
            for tt in range(NT):
                for oc in range(2):
                    o_ps = pa.tile([128, 512], F32, tag="pa", name="ops")
                    for r in range(NJ):
                        nc.tensor.matmul(
                            o_ps,
                            y_p[:, r, tt * 128:(tt + 1) * 128],
                            wo_sb[:, r, oc * 512:(oc + 1) * 512],
                            start=(r == 0),
                            stop=(r == NJ - 1 and not with_bias),
                        )
                    if with_bias:
                        nc.tensor.matmul(
                            o_ps,
                            ones_sb[:, :128],
                            bo_sb[:, oc * 512:(oc + 1) * 512],
                            start=False, stop=True,
                        )
                    o_sb = p_out.tile([128, 512], F32, tag="outs", name="osb")
                    nc.scalar.copy(out=o_sb, in_=o_ps)
                    nc.sync.dma_start(
                        out=out.ap()[t0 + tt * 128:t0 + (tt + 1) * 128,
                                     oc * 512:(oc + 1) * 512],
                        in_=o_sb,
                    )

        for blk in range(nblk):
            t0 = blk * TB
            tsl = slice(t0, t0 + TB)
            par = blk % 2
            pton = slice(par * TB, (par + 1) * TB)

            # ---- input DMAs (split across queues; block 0 chunked per c so
            # the first matmuls start after 128KB instead of 1MB) ----
            if par == 0:
                q_in = p_inq.tile([128, NJ, 2 * TB], DT, tag="qin", name="qin")
                if blk == 0:
                    for g in range(4):
                        cs = slice(2 * g, 2 * g + 2)
                        nc.sync.dma_start(out=q_in[:, cs, :],
                                          in_=qT_v[:, cs, t0:t0 + 2 * TB])
                else:
                    nc.sync.dma_start(out=q_in, in_=qT_v[:, :, t0:t0 + 2 * TB])
            k_in = [p_ink.tile([128, NJ, 2, TB], DT, tag="kin", name="kin")
                    for _ in range(2)]
            if blk == 0:
                for c in range(NJ):
                    nc.scalar.dma_start(out=k_in[0][:, c],
                                        in_=kT_v[0, :, c, :, tsl])
                    nc.gpsimd.dma_start(out=k_in[1][:, c],
                                        in_=kT_v[1, :, c, :, tsl])
                late_consts()
            else:
                nc.scalar.dma_start(out=k_in[0], in_=kT_v[0, :, :, :, tsl])
                nc.gpsimd.dma_start(out=k_in[1], in_=kT_v[1, :, :, :, tsl])

            if par == 0:
                qp = p_qp.tile([128, NJ, 2 * TB], DT, tag="qp", name="qp")
            prod = p_prod.tile([128, NJ, M, TB], DT, tag="prod", name="prod")
            ksq = p_ksq.tile([128, NJ, M, TB], DT, tag="ksq", name="ksq")
            y = p_y.tile([128, NJ, TB], DT, tag="y", name="y")

            # ---- projections (PE), prev-block tail interleaved per r ----
            for r in range(NJ):
                if par == 0 and blk > 0:
                    # q for BOTH token blocks of this pair in one N=512 run
                    q_ps = pa.tile([128, 512], F32, tag="pa", name="qps")
                    for c in range(NJ):
                        nc.tensor.matmul(
                            q_ps,
                            wq_sb[:, r, c, :],
                            q_in[:, c, :],
                            start=(c == 0),
                            stop=(c == NJ - 1 and not with_bias),
                        )
                    if with_bias:
                        nc.tensor.matmul(
                            q_ps, bq_sb[:, r * 128:(r + 1) * 128], ones_sb,
                            start=False, stop=True,
                        )
                    nc.scalar.copy(out=qp[:, r, :], in_=q_ps)

                ctt = (p_ct.tile([128, M, TB], DT, tag="ct", name="ctt")
                       if prev is not None else None)
                k_pss = []
                for pr in range(2):
                    ms = slice(2 * pr, 2 * pr + 2)
                    k_ps = pk.tile([128, 2, TB], F32, tag="pk", name="kps",
                                   padded_shape=[128, 2, 256])
                    for c in range(NJ):
                        nc.tensor.matmul(
                            k_ps,
                            wk_sb[:, r, c, :],
                            k_in[pr][:, c, :, :],
                            start=(c == 0),
                            stop=(c == NJ - 1 and not with_bias),
                        )
                    if with_bias:
                        nc.tensor.matmul(
                            k_ps, bk_sb[:, r * 128:(r + 1) * 128],
                            _bcast(ones_sb[:, :TB], 2, 1),
                            start=False, stop=True,
                        )
                    k_pss.append(k_ps)
                    if blk > 0:
                        nc.vector.tensor_mul(
                            prod[:, r, ms, :],
                            _bcast(qp[:, r, pton], 2, 1), k_ps)
                    nc.scalar.square(ksq[:, r, ms, :], k_ps)
                if blk == 0:
                    # first block: q emitted after k so the PE starts on the
                    # k pair whose DMA queue lands first; the prod muls must
                    # follow the qp write (deps derive from emission order)
                    q_ps = pa.tile([128, 512], F32, tag="pa", name="qps")
                    for c in range(NJ):
                        nc.tensor.matmul(
                            q_ps,
                            wq_sb[:, r, c, :],
                            q_in[:, c, :],
                            start=(c == 0),
                            stop=(c == NJ - 1 and not with_bias),
                        )
                    if with_bias:
                        nc.tensor.matmul(
                            q_ps, bq_sb[:, r * 128:(r + 1) * 128], ones_sb,
                            start=False, stop=True,
                        )
                    nc.scalar.copy(out=qp[:, r, :], in_=q_ps)
                    for pr in range(2):
                        ms = slice(2 * pr, 2 * pr + 2)
                        nc.vector.tensor_mul(
                            prod[:, r, ms, :],
                            _bcast(qp[:, r, pton], 2, 1), k_pss[pr])
                # prev block's weight-broadcast + ct/y chase the proj stream
                if prev is not None:
                    emit_tail_pair(prev, r, 0, ctt)
                    emit_tail_pair(prev, r, 1, ctt)
                    emit_tail_adds(prev, r, ctt, nc.gpsimd)

            # ---- attention logits, head-major [16h, t], accum over r ----
            e_sb = p_sm.tile([128, M, TB], F32, tag="e", name="e")
            for pr in range(2):
                ms = slice(2 * pr, 2 * pr + 2)
                lg = pa.tile([128, 2, TB], F32, tag="pa", name="lg")
                for r in range(NJ):
                    nc.tensor.matmul(
                        lg,
                        indl_sb[:, r, :],
                        prod[:, r, ms, :],
                        start=(r == 0),
                        stop=(r == NJ - 1),
                    )
                nc.scalar.activation(e_sb[:, ms, :], lg, func=Exp)

            # ---- softmax over M (DVE, 16 partitions) ----
            s01 = p_sm.tile([128, TB], F32, tag="s01", name="s01")
            s23 = p_sm.tile([128, TB], F32, tag="s23", name="s23")
            s = p_sm.tile([128, TB], F32, tag="s", name="s")
            rcp = p_sm.tile([128, TB], F32, tag="rcp", name="rcp")
            w_sb = p_sm.tile([128, M, TB], DT, tag="w", name="w")
            nc.gpsimd.tensor_add(s01, e_sb[:, 0, :], e_sb[:, 1, :])
            nc.gpsimd.tensor_add(s23, e_sb[:, 2, :], e_sb[:, 3, :])
            nc.gpsimd.tensor_add(s, s01, s23)
            nc.vector.reciprocal_approx_fast(rcp, s)
            nc.gpsimd.tensor_mul(w_sb, e_sb, _bcast(rcp, M, 1))

            # ---- output projection of the PREVIOUS block (PE) ----
            if prev is not None:
                emit_outproj(prev, blk - 1)

            prev = (ksq, w_sb, y)

        # ---- drain: tail of the last block (adds on DVE for fast drain) ----
        for r in range(NJ):
            ctt = p_ct.tile([128, M, TB], DT, tag="ct", name="ctt")
            emit_tail_pair(prev, r, 0, ctt)
            emit_tail_pair(prev, r, 1, ctt)
            emit_tail_adds(prev, r, ctt, nc.vector)
        emit_outproj(prev, nblk - 1)

    nc.compile()
    return nc


def _host_prep(query, key, Wq, Wk, Wo, bq, bk, bo):
    query, key = np.asarray(query), np.asarray(key)
    Wq, Wk, Wo = np.asarray(Wq), np.asarray(Wk), np.asarray(Wo)
    bq, bk, bo = np.asarray(bq), np.asarray(bk), np.asarray(bo)
    qT = np.ascontiguousarray(query.reshape(T_TOTAL, C).T).astype(NPDT)
    # [pair, c-chunk, m-in-pair, partition, t]
    kT = (key.reshape(T_TOTAL, M, C).transpose(1, 2, 0)
          .reshape(2, 2, NJ, 128, T_TOTAL).transpose(0, 2, 1, 3, 4))
    kT = np.ascontiguousarray(kT).astype(NPDT)

    def w_shuf(W):
        # [c_in, j_out] -> [p, r, c, j] with c_in = c*128+p, j_out = r*128+j
        return np.ascontiguousarray(
            W.T.reshape(NJ, 128, NJ, 128).transpose(1, 2, 0, 3)).astype(NPDT)

    wqT = w_shuf(Wq)
    wkT = w_shuf(Wk)
    woT = np.ascontiguousarray(Wo.T).astype(NPDT)

    # indl[p, r, kc] = SCALE * 1[(kc % 16) == 2r + (p>=64)] : logits
    # indicator (lhsT), full 128 output columns = 8 replicas of the 16
    # head-logit rows (full-width stationaries keep LDWEIGHTS overlapped)
    p = np.arange(128)[:, None, None]
    r = np.arange(NJ)[None, :, None]
    kc = np.arange(128)[None, None, :]
    indl = ((kc % H) == 2 * r + (p >= 64)).astype(NPDT) * NPDT(SCALE)
    # indb[k, r, p] = 1[k == 2r + (p>=64)] : broadcast indicator (lhsT);
    # rows 16..127 are zero so the replicated softmax rows are ignored
    kk = np.arange(128)[:, None, None]
    rr = np.arange(NJ)[None, :, None]
    pp = np.arange(128)[None, None, :]
    indb = (kk == 2 * rr + (pp >= 64)).astype(NPDT)

    with_bias = bool(np.any(bq) or np.any(bk) or np.any(bo))
    common = {"wqT": wqT, "wkT": wkT, "woT": woT, "indl": indl, "indb": indb}
    if with_bias:
        common |= {
            "bq": bq.reshape(1, C).astype(NPDT),
            "bk": bk.reshape(1, C).astype(NPDT),
            "bo": bo.reshape(1, C).astype(NPDT),
        }
    in_maps = []
    for i in range(NCORES):
        sl = slice(i * T_CORE, (i + 1) * T_CORE)
        in_maps.append(
            {
                "qT": np.ascontiguousarray(qT[:, sl]),
                "kT": np.ascontiguousarray(kT[:, :, :, :, sl]),
                **common,
            }
        )
    return in_maps, with_bias


_NC_CACHE = {}
_LAST_RESULT = None


def kernel(query, key, gate, Wq, bq, Wk, bk, Wv, bv, Wg, bg, Wo, bo):
    in_maps, with_bias = _host_prep(query, key, Wq, Wk, Wo, bq, bk, bo)
    key_ = (T_CORE, with_bias)
    if key_ not in _NC_CACHE:
        _NC_CACHE[key_] = build_nc(T_CORE, with_bias)
    nc = _NC_CACHE[key_]
    res = run_bass_kernel_spmd(nc, in_maps, list(range(NCORES)))
    global _LAST_RESULT
    _LAST_RESULT = res
    out = np.concatenate([res.results[i]["out"] for i in range(NCORES)], axis=0)
    return out.reshape(B, N, C)


# revision 34
# speedup vs baseline: 1.0099x; 1.0099x over previous
"""Trainium2 Bass kernel for GatedCrossAttention (B=4, N=4096, C=1024, H=16, M=4).

Reference math (dead code removed: the v/gate projections are overwritten
by views of k in the original module, so v = g = k):
    q = query @ Wq.T + bq                    [B,N,C]   -> [B,N,H,hd]
    k = key   @ Wk.T + bk                    [B,N,M,C] -> [B,N,M,H,hd]
    attn = softmax_M(SCALE * einsum('bnhc,bnmhc->bnmh', q, k))
    out  = einsum('bnmh,bnmhc->bnhc', attn, k*k) . reshape(B,N,C)
    out  = out @ Wo.T + bo

Strategy: pure data parallel over the 16384 tokens (8 cores x 2048), no
collectives.  Channels live on partitions, tokens on the free axis, so every
matmul contraction is a natural PE op.

What got it from the 652us baseline to ~435us:
  * All matmuls stream N=512 moving columns (per-MM overhead is ~24ns
    regardless of N): k-proj streams an m-pair per MM, q-proj batches two
    token blocks per MM, logits/broadcast stream m-pairs.
  * Logits via a SHARED stationary indicator indl[128,128] per r-tile,
    landing logits head-major (16 head rows, replicated x8 so the
    stationary is full-width - partial-row stationaries serialize
    LDWEIGHTS, +95ns/MM) in PSUM with 8-way accumulation over r-tiles.
    No transposes, no tiny-N matmuls.
  * Softmax weights return to channel-major via indb[128,128] stationary
    matmuls interleaved into the NEXT block's projection loop, so the PE
    stream never waits on the softmax chain (keeps the HAM clock gate at
    2.4GHz; the baseline oscillated to 1.2GHz every block).
  * qp*kp (DVE) and kp^2 (ScalarE) are computed straight out of the
    projection PSUM banks; the m-tree-adds run on the otherwise idle
    GpSimd engine; PSUM is split 3/2/3 banks across q+logits+out / k /
    broadcast pools.
  * Inputs ride three DMA queues (sync/scalar/gpsimd), weights are
    chunked per r-tile in a host-shuffled layout that keeps every DMA row
    at 2KB, and block 0's inputs are chunked per c so the first matmul
    issues ~14us in instead of ~30us.
Accumulation stays f32 in PSUM; activations/weights fp16 (rel err ~6e-4).
"""

import dataclasses
import numpy as np
from contextlib import ExitStack

try:
    import concourse.bass as bass
except ImportError:  # path fallback for bare containers
    import sys

    sys.path.insert(0, "/opt/trn_rl_repo")
    import concourse.bass as bass

import concourse.tile as tile
from concourse import bacc, mybir
from concourse.bass_utils import run_bass_kernel_spmd

# problem constants (hardcoded per the task contract)
B, N, C, H, HD, M = 4, 4096, 1024, 16, 64, 4
SCALE = float(HD) ** -0.5
NCORES = 8
T_TOTAL = B * N
T_CORE = T_TOTAL // NCORES  # 2048
TB = 256                    # tokens per block
NJ = C // 128               # 8 channel tiles (r/c run over these)
NT = TB // 128              # 2 token subtiles per block (for out proj)

DT = mybir.dt.float16
NPDT = np.float16
F32 = mybir.dt.float32
Exp = mybir.ActivationFunctionType.Exp


def _bcast(ap, reps, axis):
    """Insert a 0-stride dim of size `reps` at AP position `axis` (0=partition)."""
    new = list(ap.ap)
    new.insert(axis, [0, reps])
    return dataclasses.replace(ap, ap=new)


def build_nc(t_core=T_CORE, with_bias=False):
    nblk = t_core // TB
    assert nblk % 2 == 0
    nc = bacc.Bacc("TRN2", target_bir_lowering=False, debug=False)

    qT = nc.declare_dram_parameter("qT", [C, t_core], DT, isOutput=False)
    # k pre-shuffled on host to [pair, c-chunk, m-in-pair, partition, t] so a
    # per-pair load is a 3-dim DMA landing [p, c, m, t] in SBUF
    kT = nc.declare_dram_parameter("kT", [2, NJ, 2, 128, t_core], DT,
                                   isOutput=False)
    # wq/wk pre-shuffled on host to [p, r, c, j] so a per-r chunk is a
    # contiguous 2KB-per-partition DMA (256B rows would run at ~36GB/s)
    wqT = nc.declare_dram_parameter("wqT", [128, NJ, NJ, 128], DT,
                                    isOutput=False)
    wkT = nc.declare_dram_parameter("wkT", [128, NJ, NJ, 128], DT,
                                    isOutput=False)
    woT = nc.declare_dram_parameter("woT", [C, C], DT, isOutput=False)
    indl = nc.declare_dram_parameter("indl", [128, NJ, 128], DT, isOutput=False)
    indb = nc.declare_dram_parameter("indb", [128, NJ, 128], DT, isOutput=False)
    if with_bias:
        bq = nc.declare_dram_parameter("bq", [1, C], DT, isOutput=False)
        bk = nc.declare_dram_parameter("bk", [1, C], DT, isOutput=False)
        bo = nc.declare_dram_parameter("bo", [1, C], DT, isOutput=False)
    out = nc.declare_dram_parameter("out", [t_core, C], F32, isOutput=True)

    # DRAM views: channel dim split into (chunk, partition)
    qT_v = qT.ap().rearrange("(c p) t -> p c t", p=128)
    kT_v = kT.ap().rearrange("w c m p t -> w p c m t")
    wo_v = woT.ap().rearrange("(c p) j -> p c j", p=128)

    with tile.TileContext(nc) as tc, ExitStack() as ctx:
        consts = ctx.enter_context(tc.tile_pool(name="consts", bufs=1))
        p_inq = ctx.enter_context(tc.tile_pool(name="inq", bufs=2))
        p_ink = ctx.enter_context(tc.tile_pool(name="ink", bufs=4))
        p_qp = ctx.enter_context(tc.tile_pool(name="qp", bufs=2))
        p_prod = ctx.enter_context(tc.tile_pool(name="prod", bufs=1))
        p_ksq = ctx.enter_context(tc.tile_pool(name="ksq", bufs=2))
        p_sm = ctx.enter_context(tc.tile_pool(name="sm", bufs=2))
        p_ct = ctx.enter_context(tc.tile_pool(name="ct", bufs=2))
        p_y = ctx.enter_context(tc.tile_pool(name="y", bufs=2))
        p_out = ctx.enter_context(tc.tile_pool(name="outs", bufs=4))
        # PSUM (8 banks): pa = q-acc/logits-acc/outproj-acc, pk = k-acc,
        # pw = softmax-weight broadcast
        pa = ctx.enter_context(tc.tile_pool(name="pa", bufs=3, space="PSUM"))
        pk = ctx.enter_context(tc.tile_pool(name="pk", bufs=2, space="PSUM"))
        pw = ctx.enter_context(tc.tile_pool(name="pw", bufs=3, space="PSUM"))

        # ---- weights / constants, chunked per r-tile across DMA queues so
        # the first projection can issue ~6us in: r0 chunks ride ahead of
        # the inputs on sync/vector; the rest trail on gpsimd behind the
        # first k-pair load ----
        wq_sb = consts.tile([128, NJ, NJ, 128], DT)  # [p, r, c, j]
        wk_sb = consts.tile([128, NJ, NJ, 128], DT)
        wo_sb = consts.tile([128, NJ, C], DT)

        def w_chunk(r, eng):
            eng.dma_start(out=wq_sb[:, r], in_=wqT.ap()[:, r])
            eng.dma_start(out=wk_sb[:, r], in_=wkT.ap()[:, r])

        w_chunk(0, nc.sync)
        indl_sb = consts.tile([128, NJ, 128], DT)
        indb_sb = consts.tile([128, NJ, 128], DT)

        def late_consts():
            # spread the remaining weight chunks over all three DMA queues so
            # no queue delays the next block's inputs by a multi-us lump
            for r, eng in [(1, nc.scalar), (2, nc.gpsimd), (3, nc.sync),
                           (4, nc.scalar), (5, nc.gpsimd), (6, nc.sync),
                           (7, nc.scalar)]:
                w_chunk(r, eng)
            nc.gpsimd.dma_start(out=indl_sb, in_=indl.ap())
            nc.gpsimd.dma_start(out=indb_sb, in_=indb.ap())
            nc.sync.dma_start(out=wo_sb, in_=wo_v)
            if with_bias:
                nc.sync.dma_start(out=bq_sb, in_=bq.ap())
                nc.sync.dma_start(out=bk_sb, in_=bk.ap())
                nc.sync.dma_start(out=bo_sb, in_=bo.ap())

        if with_bias:
            ones_sb = consts.tile([1, 2 * TB], DT)
            nc.vector.memset(ones_sb, 1.0)
            bq_sb = consts.tile([1, C], DT)
            bk_sb = consts.tile([1, C], DT)
            bo_sb = consts.tile([1, C], DT)
        else:
            ones_sb = bq_sb = bk_sb = bo_sb = None

        # cross-block pipeline state: (ksq, w_sb, y) of the previous block
        prev = None
        qp = None
        q_in = None

        def emit_tail_pair(pv, r, pr, ctt):
            """PE broadcast MM of one m-pair of the prev block + chasing
            DVE ct mul.  Emitted right after this r-iter's k-pair group so
            the DVE consumer runs ~2us ahead of the slot's next writer."""
            ksq_p, w_p, _ = pv
            ms = slice(2 * pr, 2 * pr + 2)
            wb = pw.tile([128, 2, TB], F32, tag="pw", name="wb",
                         padded_shape=[128, 2, 256])
            nc.tensor.matmul(wb, indb_sb[:, r, :],
                             w_p[:, ms, :], start=True, stop=True)
            nc.vector.tensor_mul(ctt[:, ms, :], wb, ksq_p[:, r, ms, :])

        def emit_tail_adds(pv, r, ctt, eng):
            # tree-add on the otherwise idle GpSimd engine (SBUF-only ops)
            _, _, y_p = pv
            a01 = p_ct.tile([128, TB], DT, tag="a01", name="a01")
            a23 = p_ct.tile([128, TB], DT, tag="a23", name="a23")
            eng.tensor_add(a01, ctt[:, 0, :], ctt[:, 1, :])
            eng.tensor_add(a23, ctt[:, 2, :], ctt[:, 3, :])
            eng.tensor_add(y_p[:, r, :], a01, a23)

        def emit_outproj(pv, blk):
            _, _, y_p = pv
            t0 = blk * TB
            for tt in range(NT):
                for oc in range(2):
                    o_ps = pa.tile([128, 512], F32, tag="pa", name="ops")
                    for r in range(NJ):
                        nc.tensor.matmul(
                            o_ps,
                            y_p[:, r, tt * 128:(tt + 1) * 128],
                            wo_sb[:, r, oc * 512:(oc + 1) * 512],
                            start=(r == 0),
                            stop=(r == NJ - 1 and not with_bias),
                        )
                    if with_bias:
                        nc.tensor.matmul(
                            o_ps,
                            ones_sb[:, :128],
                            bo_sb[:, oc * 512:(oc + 1) * 512],
                            start=False, stop=True,
                        )
                    o_sb = p_out.tile([128, 512], F32, tag="outs", name="osb")
                    nc.scalar.copy(out=o_sb, in_=o_ps)
                    nc.sync.dma_start(
                        out=out.ap()[t0 + tt * 128:t0 + (tt + 1) * 128,
                                     oc * 512:(oc + 1) * 512],
                        in_=o_sb,
                    )

        for blk in range(nblk):
            t0 = blk * TB
            tsl = slice(t0, t0 + TB)
            par = blk % 2
            pton = slice(par * TB, (par + 1) * TB)

            # ---- input DMAs (split across queues; block 0 chunked per c so
            # the first matmuls start after 128KB instead of 1MB) ----
            if par == 0:
                q_in = p_inq.tile([128, NJ, 2 * TB], DT, tag="qin", name="qin")
                if blk == 0:
                    for g in range(4):
                        cs = slice(2 * g, 2 * g + 2)
                        nc.sync.dma_start(out=q_in[:, cs, :],
                                          in_=qT_v[:, cs, t0:t0 + 2 * TB])
                else:
                    nc.sync.dma_start(out=q_in, in_=qT_v[:, :, t0:t0 + 2 * TB])
            k_in = [p_ink.tile([128, NJ, 2, TB], DT, tag="kin", name="kin")
                    for _ in range(2)]
            if blk == 0:
                for c in range(NJ):
                    nc.scalar.dma_start(out=k_in[0][:, c],
                                        in_=kT_v[0, :, c, :, tsl])
                    nc.gpsimd.dma_start(out=k_in[1][:, c],
                                        in_=kT_v[1, :, c, :, tsl])
                late_consts()
            else:
                nc.scalar.dma_start(out=k_in[0], in_=kT_v[0, :, :, :, tsl])
                nc.gpsimd.dma_start(out=k_in[1], in_=kT_v[1, :, :, :, tsl])

            if par == 0:
                qp = p_qp.tile([128, NJ, 2 * TB], DT, tag="qp", name="qp")
            prod = p_prod.tile([128, NJ, M, TB], DT, tag="prod", name="prod")
            ksq = p_ksq.tile([128, NJ, M, TB], DT, tag="ksq", name="ksq")
            y = p_y.tile([128, NJ, TB], DT, tag="y", name="y")

            # ---- projections (PE), prev-block tail interleaved per r ----
            for r in range(NJ):
                if par == 0 and blk > 0:
                    # q for BOTH token blocks of this pair in one N=512 run
                    q_ps = pa.tile([128, 512], F32, tag="pa", name="qps")
                    for c in range(NJ):
                        nc.tensor.matmul(
                            q_ps,
                            wq_sb[:, r, c, :],
                            q_in[:, c, :],
                            start=(c == 0),
                            stop=(c == NJ - 1 and not with_bias),
                        )
                    if with_bias:
                        nc.tensor.matmul(
                            q_ps, bq_sb[:, r * 128:(r + 1) * 128], ones_sb,
                            start=False, stop=True,
                        )
                    nc.scalar.copy(out=qp[:, r, :], in_=q_ps)

                ctt = (p_ct.tile([128, M, TB], DT, tag="ct", name="ctt")
                       if prev is not None else None)
                k_pss = []
                for pr in range(2):
                    ms = slice(2 * pr, 2 * pr + 2)
                    k_ps = pk.tile([128, 2, TB], F32, tag="pk", name="kps",
                                   padded_shape=[128, 2, 256])
                    for c in range(NJ):
                        nc.tensor.matmul(
                            k_ps,
                            wk_sb[:, r, c, :],
                            k_in[pr][:, c, :, :],
                            start=(c == 0),
                            stop=(c == NJ - 1 and not with_bias),
                        )
                    if with_bias:
                        nc.tensor.matmul(
                            k_ps, bk_sb[:, r * 128:(r + 1) * 128],
                            _bcast(ones_sb[:, :TB], 2, 1),
                            start=False, stop=True,
                        )
                    k_pss.append(k_ps)
                    if blk > 0:
                        nc.vector.tensor_mul(
                            prod[:, r, ms, :],
                            _bcast(qp[:, r, pton], 2, 1), k_ps)
                    nc.scalar.square(ksq[:, r, ms, :], k_ps)
                if blk == 0:
                    # first block: q emitted after k so the PE starts on the
                    # k pair whose DMA queue lands first; the prod muls must
                    # follow the qp write (deps derive from emission order)
                    q_ps = pa.tile([128, 512], F32, tag="pa", name="qps")
                    for c in range(NJ):
                        nc.tensor.matmul(
                            q_ps,
                            wq_sb[:, r, c, :],
                            q_in[:, c, :],
                            start=(c == 0),
                            stop=(c == NJ - 1 and not with_bias),
                        )
                    if with_bias:
                        nc.tensor.matmul(
                            q_ps, bq_sb[:, r * 128:(r + 1) * 128], ones_sb,
                            start=False, stop=True,
                        )
                    nc.scalar.copy(out=qp[:, r, :], in_=q_ps)
                    for pr in range(2):
                        ms = slice(2 * pr, 2 * pr + 2)
                        nc.vector.tensor_mul(
                            prod[:, r, ms, :],
                            _bcast(qp[:, r, pton], 2, 1), k_pss[pr])
                # prev block's weight-broadcast + ct/y chase the proj stream
                if prev is not None:
                    emit_tail_pair(prev, r, 0, ctt)
                    emit_tail_pair(prev, r, 1, ctt)
                    emit_tail_adds(prev, r, ctt, nc.gpsimd)

            # ---- attention logits, head-major [16h, t], accum over r ----
            e_sb = p_sm.tile([128, M, TB], F32, tag="e", name="e")
            for pr in range(2):
                ms = slice(2 * pr, 2 * pr + 2)
                lg = pa.tile([128, 2, TB], F32, tag="pa", name="lg")
                for r in range(NJ):
                    nc.tensor.matmul(
                        lg,
                        indl_sb[:, r, :],
                        prod[:, r, ms, :],
                        start=(r == 0),
                        stop=(r == NJ - 1),
                    )
                nc.scalar.activation(e_sb[:, ms, :], lg, func=Exp)

            # ---- softmax over M (DVE, 16 partitions) ----
            s01 = p_sm.tile([128, TB], F32, tag="s01", name="s01")
            s23 = p_sm.tile([128, TB], F32, tag="s23", name="s23")
            s = p_sm.tile([128, TB], F32, tag="s", name="s")
            rcp = p_sm.tile([128, TB], F32, tag="rcp", name="rcp")
            w_sb = p_sm.tile([128, M, TB], DT, tag="w", name="w")
            nc.gpsimd.tensor_add(s01, e_sb[:, 0, :], e_sb[:, 1, :])
            nc.gpsimd.tensor_add(s23, e_sb[:, 2, :], e_sb[:, 3, :])
            nc.gpsimd.tensor_add(s, s01, s23)
            nc.vector.reciprocal_approx_fast(rcp, s)
            nc.gpsimd.tensor_mul(w_sb, e_sb, _bcast(rcp, M, 1))

            # ---- output projection of the PREVIOUS block (PE) ----
            if prev is not None:
                emit_outproj(prev, blk - 1)

            prev = (ksq, w_sb, y)

        # ---- drain: tail of the last block (adds on DVE for fast drain) ----
        for r in range(NJ):
            ctt = p_ct.tile([128, M, TB], DT, tag="ct", name="ctt")
            emit_tail_pair(prev, r, 0, ctt)
            emit_tail_pair(prev, r, 1, ctt)
            emit_tail_adds(prev, r, ctt, nc.vector)
        emit_outproj(prev, nblk - 1)

    nc.compile()
    return nc


def _host_prep(query, key, Wq, Wk, Wo, bq, bk, bo):
    query, key = np.asarray(query), np.asarray(key)
    Wq, Wk, Wo = np.asarray(Wq), np.asarray(Wk), np.asarray(Wo)
    bq, bk, bo = np.asarray(bq), np.asarray(bk), np.asarray(bo)
    qT = np.ascontiguousarray(query.reshape(T_TOTAL, C).T).astype(NPDT)
    # [pair, c-chunk, m-in-pair, partition, t]
    kT = (key.reshape(T_TOTAL, M, C).transpose(1, 2, 0)
          .reshape(2, 2, NJ, 128, T_TOTAL).transpose(0, 2, 1, 3, 4))
    kT = np.ascontiguousarray(kT).astype(NPDT)

    def w_shuf(W):
        # [c_in, j_out] -> [p, r, c, j] with c_in = c*128+p, j_out = r*128+j
        return np.ascontiguousarray(
            W.T.reshape(NJ, 128, NJ, 128).transpose(1, 2, 0, 3)).astype(NPDT)

    wqT = w_shuf(Wq)
    wkT = w_shuf(Wk)
    woT = np.ascontiguousarray(Wo.T).astype(NPDT)

    # indl[p, r, kc] = SCALE * 1[(kc % 16) == 2r + (p>=64)] : logits
    # indicator (lhsT), full 128 output columns = 8 replicas of the 16
    # head-logit rows (full-width stationaries keep LDWEIGHTS overlapped)
    p = np.arange(128)[:, None, None]
    r = np.arange(NJ)[None, :, None]
    kc = np.arange(128)[None, None, :]
    indl = ((kc % H) == 2 * r + (p >= 64)).astype(NPDT) * NPDT(SCALE)
    # indb[k, r, p] = 1[k == 2r + (p>=64)] : broadcast indicator (lhsT);
    # rows 16..127 are zero so the replicated softmax rows are ignored
    kk = np.arange(128)[:, None, None]
    rr = np.arange(NJ)[None, :, None]
    pp = np.arange(128)[None, None, :]
    indb = (kk == 2 * rr + (pp >= 64)).astype(NPDT)

    with_bias = bool(np.any(bq) or np.any(bk) or np.any(bo))
    common = {"wqT": wqT, "wkT": wkT, "woT": woT, "indl": indl, "indb": indb}
    if with_bias:
        common |= {
            "bq": bq.reshape(1, C).astype(NPDT),
            "bk": bk.reshape(1, C).astype(NPDT),
            "bo": bo.reshape(1, C).astype(NPDT),
        }
    in_maps = []
    for i in range(NCORES):
        sl = slice(i * T_CORE, (i + 1) * T_CORE)
        in_maps.append(
            {
                "qT": np.ascontiguousarray(qT[:, sl]),
                "kT": np.ascontiguousarray(kT[:, :, :, :, sl]),
                **common,
            }
        )
    return in_maps, with_bias


_NC_CACHE = {}
_LAST_RESULT = None


def kernel(query, key, gate, Wq, bq, Wk, bk, Wv, bv, Wg, bg, Wo, bo):
    in_maps, with_bias = _host_prep(query, key, Wq, Wk, Wo, bq, bk, bo)
    key_ = (T_CORE, with_bias)
    if key_ not in _NC_CACHE:
        _NC_CACHE[key_] = build_nc(T_CORE, with_bias)
    nc = _NC_CACHE[key_]
    res = run_bass_kernel_spmd(nc, in_maps, list(range(NCORES)))
    global _LAST_RESULT
    _LAST_RESULT = res
    out = np.concatenate([res.results[i]["out"] for i in range(NCORES)], axis=0)
    return out.reshape(B, N, C)
